# revision 46
# baseline (speedup 1.0000x reference)
"""Trainium2 Bass kernel: 16-head causal attention with sink logit.

Contract: kernel(**inputs) takes the FULL inputs of the reference
(x [2,2048,1024], W_Q/W_K/W_V/W_out [1024,1024], sink [16]) and returns
the FULL output [2,2048,1024], running on 8 NeuronCores.

Sharding: core c = b*4 + g handles batch b and heads [4g, 4g+4).
Each core computes yT_partial [1024, 2048] = W_out_slice^T @ attn^T;
host sums the 4 partials per batch and transposes.

v6: bf16 activations/weights throughout (f32 PSUM accumulation);
QKV projections spread piecewise through the attention kt loop so the
scalar-engine exp stream never starves; causally-trimmed diagonal
score/PV/exp blocks with one 128x128 triangular mask; sink folded into
the PSUM accumulation via tiny PE matmuls; normalization broadcasts
1/denom with a PE matmul and stages it to SBUF via the scalar engine's
Copy so the DVE multiply reads only one PSUM operand; causal-mask
multiplies run on the otherwise-idle GPSIMD engine.
"""

import sys
import numpy as np

if "/opt/trn_rl_repo" not in sys.path:
    sys.path.insert(0, "/opt/trn_rl_repo")

B, T, C = 2, 2048, 1024
H, D = 16, 64
G = 4                # heads per core
DH = G * D           # 256 head-dims per core
NCORES = 8
QC = 512             # q chunk (matmul moving free dim)
NQ = T // QC         # 4
NKT = T // 128       # 16 k-tiles
NCC = C // 128       # 8 contraction chunks over C
SCALE = 1.0 / float(np.sqrt(D))

# vp_sb per-kt slot layout (386 cols per kt):
#   head0 (even): [V(64) | one]            off 0,   width 65,  denom row 64
#   head1 (odd):  [one | zeros(63) | V(64)] off 65,  width 128, denom row 0
#   head2 (even): [V(64) | one]            off 193, width 65,  denom row 64
#   head3 (odd):  [one | zeros(63) | V(64)] off 258, width 128, denom row 0
VP_W = 386
VP_OFF = [0, 65, 193, 258]


def build_program(reps=1):
    """Build the per-core Bass program. reps>1 repeats the compute body
    (same inputs -> same outputs) for differential wall-clock timing."""
    from contextlib import ExitStack

    import concourse.bass as bass
    import concourse.tile as tile
    from concourse import bacc, mybir

    f32 = mybir.dt.float32
    f32r = mybir.dt.float32r
    bf16 = mybir.dt.bfloat16
    AF = mybir.ActivationFunctionType
    Alu = mybir.AluOpType

    nc = bacc.Bacc("TRN2", target_bir_lowering=False, debug=False)

    xt_d = nc.dram_tensor("xt", [C, T], bf16, kind="ExternalInput").ap()
    wq_d = nc.dram_tensor("wq", [C, DH], bf16, kind="ExternalInput").ap()
    wk_d = nc.dram_tensor("wk", [C, DH], bf16, kind="ExternalInput").ap()
    wv_d = nc.dram_tensor("wv", [C, DH], bf16, kind="ExternalInput").ap()
    wo_d = nc.dram_tensor("wo", [DH, C], bf16, kind="ExternalInput").ap()
    sk_d = nc.dram_tensor("sk", [1, G], f32, kind="ExternalInput").ap()
    cm_d = nc.dram_tensor("cm", [128, 256], bf16, kind="ExternalInput").ap()
    vpc_d = nc.dram_tensor("vpc", [128, NKT * 65], bf16, kind="ExternalInput").ap()
    ind_d = nc.dram_tensor("ind", [128, 128], f32r, kind="ExternalInput").ap()
    onr_d = nc.dram_tensor("onr", [1, QC], bf16, kind="ExternalInput").ap()
    yt_d = nc.dram_tensor("yt", [C, T], f32, kind="ExternalOutput").ap()

    xt_v = xt_d.rearrange("(n p) m -> p n m", p=128)   # [128, 8, 2048]
    wq_v = wq_d.rearrange("(n p) m -> p n m", p=128)   # [128, 8, 256]
    wk_v = wk_d.rearrange("(n p) m -> p n m", p=128)
    wv_v = wv_d.rearrange("(n p) m -> p n m", p=128)
    wo_v = wo_d.rearrange("(n p) m -> p n m", p=128)   # [128, 2, 1024]
    yt_v = yt_d.rearrange("(n p) m -> p n m", p=128)   # [128, 8, 2048]

    with tile.TileContext(nc) as tc, ExitStack() as ctx:
        P = lambda name, bufs: ctx.enter_context(tc.tile_pool(name=name, bufs=bufs))
        const_p = P("const", 1)
        big_p = P("big", 1)
        p_p = P("p", 4)
        y_p = P("y", 2)
        row_p = P("row", 1)
        atu_p = P("atu", 2)
        ps_p = ctx.enter_context(tc.tile_pool(name="ps", bufs=3, space="PSUM"))
        o_p = ctx.enter_context(tc.tile_pool(name="o", bufs=1, space="PSUM"))

        # ---- persistent SBUF tensors ----
        xt_sb = big_p.tile([128, NCC * T], bf16, tag="xt")           # 32KB/part
        wq_sb = big_p.tile([128, NCC * DH], bf16, tag="wq")
        wk_sb = big_p.tile([128, NCC * DH], bf16, tag="wk")
        wv_sb = big_p.tile([128, NCC * DH], bf16, tag="wv")
        wo_sb = big_p.tile([128, 2 * C], bf16, tag="wo")
        qt_sb = big_p.tile([128, 2 * T], bf16, tag="qt")
        kt_sb = big_p.tile([128, 2 * T], bf16, tag="kt")
        vp_sb = big_p.tile([128, NKT * VP_W], bf16, tag="vp")
        at_sb = big_p.tile([128, 2 * T], bf16, tag="at")             # attn^T normalized
        cm_sb = const_p.tile([128, 256], bf16, tag="cm")             # tri | tri
        ind_sb = const_p.tile([128, 128], f32r, tag="ind")
        skr_sb = const_p.tile([128, G], f32, tag="skr")
        esk_sb = const_p.tile([128, G], f32, tag="esk")
        eskb_sb = const_p.tile([128, G], bf16, tag="eskb")
        ones_sb = const_p.tile([128, QC], bf16, tag="ones")

        # ---- phase 0: loads + constants ----
        # weights first (K proj blocks on wk), then x in q-chunk pieces so
        # proj(0) can start after the first 512 columns of each c-chunk land
        nc.sync.dma_start(
            wk_sb[:].rearrange("p (n m) -> p n m", m=DH), wk_v[:, :, :])
        nc.sync.dma_start(
            wq_sb[:].rearrange("p (n m) -> p n m", m=DH), wq_v[:, :, :])
        nc.sync.dma_start(
            wv_sb[:].rearrange("p (n m) -> p n m", m=DH), wv_v[:, :, :])
        nc.sync.dma_start(cm_sb[:, :], cm_d[:, :])
        nc.sync.dma_start(skr_sb[0:1, :], sk_d[:, :])
        nc.sync.dma_start(skr_sb[64:65, :], sk_d[:, :])
        for qcl in range(NQ):
            for i in range(NCC):
                nc.sync.dma_start(
                    xt_sb[:, i * T + qcl * QC: i * T + (qcl + 1) * QC],
                    xt_v[:, i, qcl * QC:(qcl + 1) * QC])
        nc.sync.dma_start(
            wo_sb[:].rearrange("p (n m) -> p n m", m=C), wo_v[:, :, :])
        nc.scalar.activation(esk_sb[0:1, :], skr_sb[0:1, :], AF.Exp)
        nc.scalar.activation(esk_sb[64:65, :], skr_sb[64:65, :], AF.Exp)
        with nc.allow_low_precision(reason="bf16 sink"):
            nc.vector.tensor_copy(eskb_sb[0:1, :], esk_sb[0:1, :])
            nc.vector.tensor_copy(eskb_sb[64:65, :], esk_sb[64:65, :])
        nc.sync.dma_start(ones_sb[0:1, :], onr_d[:, :])
        nc.sync.dma_start(ones_sb[64:65, :], onr_d[:, :])
        # vp ones columns ([1,1,0*63] pattern per region)
        vp_view = vp_sb[:].rearrange("p (k w) -> p k w", w=VP_W)
        vpc_view = vpc_d.rearrange("p (k w) -> p k w", w=65)
        nc.sync.dma_start(vp_view[:, :, 64:129], vpc_view[:, :, :])
        nc.sync.dma_start(vp_view[:, :, 257:322], vpc_view[:, :, :])
        nc.sync.dma_start(ind_sb[:, :], ind_d[:, :])

        deferred = []

        def pop():
            if deferred:
                deferred.pop(0)()

        for _ in range(reps):
            # ---- projections for one 512-wide q/k chunk, emitted piecewise
            # (yield points let attention interleave between matmul groups) ----
            def proj_steps(qc):
                for w_sb, t_sb in ((wk_sb, kt_sb), (wq_sb, qt_sb)):
                    ps = ps_p.tile([128, 2 * QC], f32, tag="ps")
                    for mt in range(2):           # head pair -> 128 d rows
                        for cig in range(2):
                            for ci in range(cig * 4, cig * 4 + 4):
                                nc.tensor.matmul(
                                    ps[:, mt * QC:(mt + 1) * QC],
                                    w_sb[:, ci * DH + mt * 128: ci * DH + (mt + 1) * 128],
                                    xt_sb[:, ci * T + qc * QC: ci * T + qc * QC + QC],
                                    start=(ci == 0), stop=(ci == NCC - 1))
                            yield
                    dst = t_sb[:].rearrange("p (n m) -> p n m", m=T)[
                        :, :, qc * QC:(qc + 1) * QC]
                    with nc.allow_low_precision(reason="bf16 qkv"):
                        nc.vector.tensor_copy(
                            dst, ps[:].rearrange("p (n m) -> p n m", m=QC))
                    yield
                # V natural [t, d] for tq=qc into padded vp layout
                ps = ps_p.tile([128, 2 * QC], f32, tag="ps")
                for sub in range(4):
                    tt = qc * 4 + sub
                    for ci in range(NCC):
                        nc.tensor.matmul(
                            ps[:, sub * DH:(sub + 1) * DH],
                            xt_sb[:, ci * T + tt * 128: ci * T + (tt + 1) * 128],
                            wv_sb[:, ci * DH: (ci + 1) * DH],
                            start=(ci == 0), stop=(ci == NCC - 1))
                    yield
                with nc.allow_low_precision(reason="bf16 v"):
                    for sub in range(4):
                        tt = qc * 4 + sub
                        base = tt * VP_W
                        s0 = sub * DH
                        nc.vector.tensor_copy(
                            vp_sb[:, base + 0: base + 64], ps[:, s0:s0 + 64])
                        nc.vector.tensor_copy(
                            vp_sb[:, base + 129: base + 257], ps[:, s0 + 64:s0 + 192])
                        nc.vector.tensor_copy(
                            vp_sb[:, base + 322: base + 386], ps[:, s0 + 192:s0 + 256])
                yield

            # ---- attention per q-chunk, software-pipelined on PE ----
            def emit_scores(p, qc, kt):
                d = kt - 4 * qc
                off = 128 * d if d > 0 else 0
                sAB = ps_p.tile([128, 2 * QC], f32, tag="ps")
                nc.tensor.matmul(
                    sAB[:, off:QC],
                    kt_sb[0:64, p * T + kt * 128: p * T + (kt + 1) * 128],
                    qt_sb[0:64, p * T + qc * QC + off: p * T + (qc + 1) * QC],
                    start=True, stop=True)
                nc.tensor.matmul(
                    sAB[:, QC + off:2 * QC],
                    kt_sb[64:128, p * T + kt * 128: p * T + (kt + 1) * 128],
                    qt_sb[64:128, p * T + qc * QC + off: p * T + (qc + 1) * QC],
                    start=True, stop=True)
                pAB = p_p.tile([128, 2 * QC], bf16, tag="p")
                sv = sAB[:].rearrange("p (h q) -> p h q", h=2)[:, :, off:QC]
                pv = pAB[:].rearrange("p (h q) -> p h q", h=2)[:, :, off:QC]
                with nc.allow_low_precision(reason="bf16 probs"):
                    nc.scalar.activation(pv, sv, AF.Exp, scale=SCALE)
                if d >= 0:
                    mv = pAB[:].rearrange("p (h q) -> p h q", h=2)[
                        :, :, off:off + 128]
                    cmv = cm_sb[:].rearrange("p (h q) -> p h q", h=2)
                    with nc.allow_low_precision(reason="0/1 mask mult"):
                        nc.gpsimd.tensor_mul(mv, mv, cmv)
                return pAB

            def emit_pv(p, qc, kt, nkt, oAB, pAB):
                d = kt - 4 * qc
                off = 128 * d if d > 0 else 0
                hA, hB = 2 * p, 2 * p + 1
                base = kt * VP_W
                nc.tensor.matmul(
                    oAB[0:65, off:QC],
                    vp_sb[:, base + VP_OFF[hA]: base + VP_OFF[hA] + 65],
                    pAB[:, off:QC],
                    start=(kt == 0), stop=False,
                    skip_group_check=True)
                nc.tensor.matmul(
                    oAB[:, QC + off:2 * QC],
                    vp_sb[:, base + VP_OFF[hB]: base + VP_OFF[hB] + 128],
                    pAB[:, QC + off:2 * QC],
                    start=(kt == 0), stop=False,
                    skip_group_check=True)

            def emit_close(p, oAB):
                # sink contribution to the denominator rows closes the
                # oAB accumulation group
                hA, hB = 2 * p, 2 * p + 1
                nc.tensor.matmul(
                    oAB[64:65, 0:QC], eskb_sb[64:65, hA:hA + 1],
                    ones_sb[64:65, :], start=False, stop=True,
                    skip_group_check=True)
                nc.tensor.matmul(
                    oAB[0:1, QC:2 * QC], eskb_sb[0:1, hB:hB + 1],
                    ones_sb[0:1, :], start=False, stop=True,
                    skip_group_check=True)

            def make_normalize(p, qc, oAB):
                def emit():
                    rc = row_p.tile([128, QC], f32r, tag="rowr")
                    bc = ps_p.tile([128, 2 * QC], f32, tag="ps")
                    bcs = atu_p.tile([128, QC], bf16, tag="atu")
                    with nc.allow_low_precision(reason="f32r recip broadcast"):
                        nc.vector.reciprocal(rc[64:65, :], oAB[64:65, 0:QC])
                        nc.vector.reciprocal(rc[0:1, :], oAB[0:1, QC:2 * QC])
                    nc.tensor.matmul(
                        bc[:, 0:QC], ind_sb[64:65, :], rc[64:65, :],
                        start=True, stop=False, skip_group_check=True)
                    nc.tensor.matmul(
                        bc[:, 0:QC], ind_sb[0:1, :], rc[0:1, :],
                        start=False, stop=True, skip_group_check=True)
                    with nc.allow_low_precision(reason="bf16 recip broadcast"):
                        nc.scalar.activation(bcs[:, :], bc[:, 0:QC], AF.Copy)
                    with nc.allow_low_precision(reason="bf16 attn out"):
                        nc.vector.tensor_mul(
                            at_sb[0:64, p * T + qc * QC: p * T + qc * QC + QC],
                            oAB[0:64, 0:QC], bcs[0:64, :])
                        nc.vector.tensor_mul(
                            at_sb[64:128, p * T + qc * QC: p * T + qc * QC + QC],
                            oAB[64:128, QC:2 * QC], bcs[64:128, :])
                return emit

            def make_wout(qc, cop):
                def emit():
                    ps = ps_p.tile([128, 2 * QC], f32, tag="ps")
                    for half in range(2):
                        co = cop * 2 + half
                        for j in range(2):
                            nc.tensor.matmul(
                                ps[:, half * QC:(half + 1) * QC],
                                wo_sb[:, j * C + co * 128: j * C + (co + 1) * 128],
                                at_sb[:, j * T + qc * QC: j * T + qc * QC + QC],
                                start=(j == 0), stop=(j == 1))
                    yt = y_p.tile([128, 2 * QC], f32, tag="y")
                    nc.vector.tensor_copy(yt[:, :], ps[:, :])
                    nc.sync.dma_start(
                        yt_v[:, cop * 2: cop * 2 + 2, qc * QC: qc * QC + QC],
                        yt[:, :].rearrange("p (n m) -> p n m", m=QC))
                return emit

            state = {"gen": None, "first": True}

            def advance(n):
                g = state["gen"]
                if g is None:
                    return False
                for _ in range(n):
                    try:
                        next(g)
                    except StopIteration:
                        state["gen"] = None
                        return False
                return True

            state["first"] = True
            for qc in range(NQ):
                if state["first"]:
                    for i, _ in enumerate(proj_steps(qc)):
                        if i == 4:
                            pop()
                            pop()
                        elif i in (9, 14):
                            pop()
                    state["first"] = False
                state["gen"] = proj_steps(qc + 1) if qc + 1 < NQ else None
                nkt = 4 * qc + 4
                for p in range(2):
                    oAB = o_p.tile([128, 2 * QC], f32, tag="o")
                    pend = []
                    for kt in range(nkt):
                        pend.append((kt, emit_scores(p, qc, kt)))
                        if state["gen"] is not None and (p == 0 or kt % 2 == 1):
                            advance(1 if p else 2)
                        elif kt >= 2:
                            pop()
                        if len(pend) > 2:
                            k0, pb = pend.pop(0)
                            emit_pv(p, qc, k0, nkt, oAB, pb)
                    for k0, pb in pend:
                        emit_pv(p, qc, k0, nkt, oAB, pb)
                    emit_close(p, oAB)
                    make_normalize(p, qc, oAB)()
                for cop in range(NCC // 2):
                    deferred.append(make_wout(qc, cop))
                while advance(1):
                    pop()
        for fn in deferred:
            fn()
        deferred.clear()

    nc.compile()
    return nc


def make_tri_mask():
    """[128, 256] bf16: upper-tri (q>=k) pattern duplicated side by side."""
    import ml_dtypes
    kl = np.arange(128)[:, None]
    ql = np.arange(128)[None, :]
    tri = (ql >= kl).astype(np.float32)
    return np.concatenate([tri, tri], axis=1).astype(ml_dtypes.bfloat16)


def shard_inputs(x, W_Q, W_K, W_V, W_out, sink):
    import ml_dtypes
    bf16 = ml_dtypes.bfloat16
    cm = make_tri_mask()
    vpc = np.zeros((128, 65), dtype=np.float32)
    vpc[:, 0:2] = 1.0
    vpc = np.tile(vpc, (1, NKT)).astype(ml_dtypes.bfloat16)
    ind = np.zeros((128, 128), dtype=np.float32)
    ind[64, 0:64] = 1.0   # head A recip (row 64) -> rows 0-63
    ind[0, 64:128] = 1.0  # head B recip (row 0) -> rows 64-127
    in_maps = []
    for c in range(NCORES):
        b, g = divmod(c, G)
        cols = slice(g * DH, (g + 1) * DH)
        in_maps.append({
            "xt": np.ascontiguousarray(x[b].T).astype(bf16),
            "wq": np.ascontiguousarray(W_Q[:, cols]).astype(bf16),
            "wk": np.ascontiguousarray(W_K[:, cols]).astype(bf16),
            "wv": np.ascontiguousarray(W_V[:, cols]).astype(bf16),
            "wo": np.ascontiguousarray(W_out[cols, :]).astype(bf16),
            "sk": np.ascontiguousarray(sink[g * G:(g + 1) * G][None, :]),
            "cm": cm,
            "vpc": vpc,
            "ind": ind,
            "onr": np.ones((1, QC), dtype=np.float32).astype(bf16),
        })
    return in_maps


def gather_outputs(results):
    out = np.zeros((B, T, C), dtype=np.float32)
    for b in range(B):
        acc = np.zeros((C, T), dtype=np.float32)
        for g in range(G):
            acc += results[b * G + g]["yt"]
        out[b] = acc.T
    return out


_CACHE = {}


def _get_program():
    if "nc" not in _CACHE:
        _CACHE["nc"] = build_program(reps=1)
    return _CACHE["nc"]


def kernel(x, W_Q, W_K, W_V, W_out, sink):
    from concourse.bass_utils import run_bass_kernel_spmd

    x = np.asarray(x, dtype=np.float32)
    W_Q = np.asarray(W_Q, dtype=np.float32)
    W_K = np.asarray(W_K, dtype=np.float32)
    W_V = np.asarray(W_V, dtype=np.float32)
    W_out = np.asarray(W_out, dtype=np.float32)
    sink = np.asarray(sink, dtype=np.float32)

    nc = _get_program()
    in_maps = shard_inputs(x, W_Q, W_K, W_V, W_out, sink)
    res = run_bass_kernel_spmd(nc, in_maps, core_ids=list(range(NCORES)))
    return gather_outputs(res.results)
